# revision 43
# baseline (speedup 1.0000x reference)
"""Trainium2 Bass kernel for DeformableConvBlock (B=4, C=64, H=W=128, K=3).

Self-contained: builds an SPMD Bass/Tile program for 8 NeuronCores.
Core c handles image c//2, output-row half c%2 (data-parallel over
batch x row-halves).

v5: gather-paced pipeline with the DMA rings reserved for the SWDGE
patch gathers. DVE does the bilinear weight multiply (corner-minor
table rows [64 ch x 4 corners] - the fast broadcast pattern) plus two
corner adds; the PE transposes the reduced 576-row contraction blocks
(matmul is_transpose) with Scalar PSUM->SBUF copies, and the conv
matmul contracts 5 x 128 tiles. The conv bias rides a constant-1 pad
row of the contraction. Index-replication copies and the offset cast
run on the Scalar engine to keep the DVE light. Out-of-bounds corners
are handled by index-clamping into the table's zero-padded border
(exactly equivalent to the reference's validity masking), so the prep
needs no mask ops.

kernel(**inputs) takes the full unsharded numpy inputs and returns the
full [4, 64, 128, 128] float32 output.
"""
from contextlib import ExitStack

import numpy as np
import ml_dtypes

import concourse.bacc as bacc
import concourse.bass as bass
import concourse.mybir as mybir
from concourse.tile import TileContext
from concourse.vector_clock import ScopedClock, VectorClock

F32 = mybir.dt.float32
BF16 = mybir.dt.bfloat16
I32 = mybir.dt.int32
I16 = mybir.dt.int16
AF = mybir.ActivationFunctionType
OP = mybir.AluOpType

H = W = 128
C = 64
O = 64
KK = 9
ROWS = 64            # output rows per core
NPX = ROWS * W       # 8192
PADT = 2             # table padding on each side
PW = W + 2 * PADT    # 132
NTAB = PW * PW       # 17424 table rows
EROW = 4 * C         # 256 elems per table row
KTILES = 19          # (k, c-half, f) = 2304 rows + bias tile
CHUNK = 2            # rows per gather/matmul chunk (32 calls keep all 4
                     # SWDGE queues saturated: 194us vs 270us at 4 rows)
CCH = 4              # rows per offset-conv chunk (PSUM 512-col limit)
BATCH = 16           # rows per prep batch
BIG = 1024.0         # floor-trick offset


class TileContextSplitDrain(TileContext):
    """Stock epilogue emits one Drain with one wait per outstanding proc;
    this walrus rejects >1 sync wait per instruction, so emit one Drain
    per proc instead."""

    def _drain_and_barrier(self, tick_clock, wait_clock):
        gc = tick_clock.global_clock
        nprocs = len(gc)
        emitted = False
        for p in range(nprocs):
            t = gc[p]
            if t <= 0:
                continue
            vec = [0] * nprocs
            vec[p] = t
            drain_inst = self.nc.sync.drain()
            wait_clock.add_sem_waits(
                drain_inst.ins, ScopedClock({None: VectorClock(vec)})
            )
            si = drain_inst.ins.sync_info
            assert si is None or len(si.on_wait) <= 1
            emitted = True
        if not emitted:
            self.nc.sync.drain()
        self.nc.all_engine_barrier()
        assert self.sems is not None
        popped = self.nc._tile_sem_poison_stack.pop()
        assert popped is self._sem_poison
        self.nc.clear_and_free_semaphores(list(self.sems.allocated().values()))
        self.nc.all_engine_barrier()


def build_program(nrows=ROWS, g_bufs=8):
    """Build the SPMD Bass program. nrows<=64 shrinks work for sim tests."""
    npx = nrows * W
    nchunks = nrows // CHUNK
    nbatch = max(1, nrows // BATCH)
    brow = min(BATCH, nrows)              # rows per prep batch
    byk = brow * KK                       # idx cols per batch
    cyk = CHUNK * KK                      # idx cols per chunk
    cpb = brow // CHUNK                   # gather chunks per batch
    ccb = brow // CCH                     # conv chunks per batch
    band = brow + 2                       # xband rows per batch

    nc = bacc.Bacc(num_swdge_queues=4)
    xband = nc.dram_tensor("xband", [C, nrows + 2, W + 2], BF16, kind="ExternalInput")
    table = nc.dram_tensor("table", [NTAB, EROW], BF16, kind="ExternalInput")
    w_off = nc.dram_tensor("w_off", [KK, C, 18], BF16, kind="ExternalInput")
    b_off = nc.dram_tensor("b_off", [18, 1], F32, kind="ExternalInput")
    w2 = nc.dram_tensor("w2", [KTILES, 128, O], BF16, kind="ExternalInput")
    cgrid = nc.dram_tensor("cgrid", [2, nrows * KK], F32, kind="ExternalInput")
    iotax = nc.dram_tensor("iotax", [128, 1], F32, kind="ExternalInput")
    ident = nc.dram_tensor("ident", [128, 128], BF16, kind="ExternalInput")
    out = nc.dram_tensor("out", [O, npx], F32, kind="ExternalOutput")

    ctx = ExitStack()
    with TileContextSplitDrain(nc) as tc:
        const_pool = ctx.enter_context(tc.tile_pool(name="const", bufs=1))
        scr_pool = ctx.enter_context(tc.tile_pool(name="scr", bufs=1))
        xbb_pool = ctx.enter_context(tc.tile_pool(name="xbb", bufs=1))
        grid_pool = ctx.enter_context(tc.tile_pool(name="grid", bufs=1))
        oc_pool = ctx.enter_context(tc.tile_pool(name="oc", bufs=1))
        op_pool = ctx.enter_context(tc.tile_pool(name="op", bufs=1))
        opf_pool = ctx.enter_context(tc.tile_pool(name="opf", bufs=1))
        wf_pool = ctx.enter_context(tc.tile_pool(name="wf", bufs=4))
        wr_pool = ctx.enter_context(tc.tile_pool(name="wr", bufs=4))
        g_pool = ctx.enter_context(tc.tile_pool(name="g", bufs=g_bufs))
        st_pool = ctx.enter_context(tc.tile_pool(name="st", bufs=2))
        o_pool = ctx.enter_context(tc.tile_pool(name="o", bufs=1))
        psum_pool = ctx.enter_context(tc.tile_pool(name="ps", bufs=2, space="PSUM"))
        pot_pool = ctx.enter_context(tc.tile_pool(name="pot", bufs=1, space="PSUM"))
        pst_pool = ctx.enter_context(tc.tile_pool(name="pst", bufs=2, space="PSUM"))
        psum2_pool = ctx.enter_context(tc.tile_pool(name="ps2", bufs=2, space="PSUM"))

        # ---- loads ----
        wof = const_pool.tile([C, KK * 18], BF16)
        wof_v = wof[:].rearrange("c (k e) -> c k e", k=KK, e=18)
        nc.sync.dma_start(out=wof_v, in_=w_off[:].rearrange("k c e -> c k e"))

        bof = const_pool.tile([18, 1], F32)
        nc.sync.dma_start(out=bof[:], in_=b_off[:])

        w2t = const_pool.tile([128, KTILES * O], BF16)
        w2t_v = w2t[:].rearrange("p (j e) -> p j e", j=KTILES, e=O)
        nc.sync.dma_start(out=w2t_v, in_=w2[:].rearrange("j p e -> p j e"))

        idt = const_pool.tile([128, 128], BF16)
        nc.sync.dma_start(out=idt[:], in_=ident[:])

        iox = const_pool.tile([128, 1], F32)
        nc.sync.dma_start(out=iox[:], in_=iotax[:])

        onesc = const_pool.tile([128, CHUNK], BF16)
        nc.vector.memset(onesc[:], 1.0)

        # prep scratch (live range = within one batch; DVE runs batches
        # sequentially so one buffer per name suffices)
        scr = {}
        for nm in ("ya", "yb", "yd", "ye", "xa", "xc", "xd", "xe"):
            scr[nm] = scr_pool.tile([128, byk], F32, name=f"scr_{nm}")
        scr_i32 = scr_pool.tile([128, byk], I32, name="scr_i32")
        scr_i16 = scr_pool.tile([128, byk], I16, name="scr_i16")

        wf_tiles = [None] * nbatch
        wr_tiles = [None] * nbatch

        # ---- stage A (per batch): offset conv + transpose + prep ----
        def emit_stage_a(b):
            xbb = xbb_pool.tile([C, band * (W + 2)], BF16, tag="xbb")
            nc.sync.dma_start(
                out=xbb[:],
                in_=xband[:, b * brow:b * brow + band, :].rearrange(
                    "c h w -> c (h w)"))
            xbb_v = xbb[:].rearrange("c (h w) -> c h w", h=band, w=W + 2)

            cyb = grid_pool.tile([128, byk], F32, tag="cy")
            nc.sync.dma_start(
                out=cyb[:],
                in_=cgrid[0:1, b * byk:(b + 1) * byk].to_broadcast((128, byk)))
            cxb = grid_pool.tile([128, byk], F32, tag="cx")
            nc.sync.dma_start(
                out=cxb[:],
                in_=cgrid[1:2, b * byk:(b + 1) * byk].to_broadcast((128, byk)))

            # offset conv, CCH rows per psum tile
            ocb = oc_pool.tile([32, brow * W], BF16, tag="ocb")
            if b == 0:
                nc.vector.memset(ocb[:], 0.0)
            for cc in range(ccb):
                r0 = cc * CCH
                ps = psum_pool.tile([18, CCH * W], F32, tag="ps1")
                for k in range(KK):
                    dy, dx = k // 3, k % 3
                    rhs = xbb_v[:, r0 + dy:r0 + dy + CCH, dx:dx + W]
                    nc.tensor.matmul(
                        out=ps[:], lhsT=wof_v[:, k, :], rhs=rhs,
                        start=(k == 0), stop=(k == KK - 1),
                    )
                nc.scalar.activation(
                    out=ocb[:18, cc * CCH * W:(cc + 1) * CCH * W], in_=ps[:],
                    func=AF.Identity, bias=bof[:],
                )
            # offsets to pixel-major via PE transpose:
            # [32, brow*W] -> [128, brow, 32]
            pot = pot_pool.tile([128, brow * 32], BF16, tag="pot")
            for bi in range(brow):
                nc.tensor.transpose(
                    out=pot[:, bi * 32:(bi + 1) * 32],
                    in_=ocb[:, bi * W:(bi + 1) * W],
                    identity=idt[0:32, 0:32])
            opb = op_pool.tile([128, brow * 32], BF16, tag="opb")
            nc.scalar.copy(out=opb[:], in_=pot[:])
            opb3 = opb[:].rearrange("p (y e) -> p y e", y=brow, e=32)

            opf = opf_pool.tile([128, brow * 18], F32, tag="opf")
            opf_v = opf[:].rearrange("p (y e) -> p y e", y=brow, e=18)
            nc.scalar.copy(out=opf_v, in_=opb3[:, :, 0:18])

            for axis in (0, 1):  # 0: y, 1: x
                a, bb, d, e = (
                    (scr["ya"], scr["yb"], scr["yd"], scr["ye"]) if axis == 0
                    else (scr["xa"], scr["xc"], scr["xd"], scr["xe"]))
                i0 = scr_i32
                off_src = opf_v[:, :, axis * 9:axis * 9 + 9]
                grid = cyb if axis == 0 else cxb
                s3 = a[:].rearrange("p (y k) -> p y k", y=brow, k=KK)
                grid3 = grid[:].rearrange("p (y k) -> p y k", y=brow, k=KK)
                nc.vector.tensor_tensor(out=s3, in0=off_src, in1=grid3, op=OP.add)
                if axis == 1:
                    nc.vector.tensor_tensor(
                        out=a[:], in0=a[:],
                        in1=iox[:].to_broadcast((128, byk)), op=OP.add)
                # floor via +BIG truncation: d = trunc(a+BIG)-BIG, then
                # correct for convert rounding mode (floor = t - (a < t))
                nc.vector.tensor_scalar_add(out=bb[:], in0=a[:], scalar1=BIG)
                nc.vector.tensor_copy(out=i0[:], in_=bb[:])
                nc.vector.tensor_copy(out=d[:], in_=i0[:])
                nc.vector.tensor_scalar_add(out=d[:], in0=d[:], scalar1=-BIG)
                nc.vector.tensor_tensor(out=e[:], in0=a[:], in1=d[:], op=OP.is_lt)
                nc.vector.tensor_tensor(out=d[:], in0=d[:], in1=e[:], op=OP.subtract)
                # fractions: b = frac, a = 1-frac; e = clamped floor for idx
                nc.vector.tensor_tensor(out=bb[:], in0=a[:], in1=d[:], op=OP.subtract)
                nc.vector.tensor_scalar(
                    out=a[:], in0=bb[:], scalar1=-1.0, scalar2=1.0,
                    op0=OP.mult, op1=OP.add)
                nc.vector.tensor_scalar(
                    out=e[:], in0=d[:], scalar1=-2.0, scalar2=128.0,
                    op0=OP.max, op1=OP.min)

            # wf[...,f=2i+j] = wy_i * wx_j   (y: a=w0 b=w1 / x: a=w0 c=w1)
            wf = wf_pool.tile([128, brow * 36], BF16, tag="wf")
            wf_v = wf[:].rearrange("p (y k f) -> p y k f", y=brow, k=KK, f=4)
            for i, wy in enumerate((scr["ya"], scr["yb"])):
                for j, wx in enumerate((scr["xa"], scr["xc"])):
                    dst = wf_v[:, :, :, 2 * i + j].rearrange("p y k -> p (y k)")
                    nc.vector.tensor_tensor(out=dst, in0=wy[:], in1=wx[:], op=OP.mult)
            wf_tiles[b] = wf

            # idx = (yc+2)*132 + (xc+2); yc = scr.ye, xc = scr.xe
            idxf = scr["yd"]
            nc.vector.tensor_scalar(
                out=idxf[:], in0=scr["ye"][:], scalar1=float(PW),
                scalar2=float(PADT * PW + PADT), op0=OP.mult, op1=OP.add)
            nc.vector.tensor_tensor(out=idxf[:], in0=idxf[:], in1=scr["xe"][:], op=OP.add)
            nc.vector.tensor_copy(out=scr_i16[:], in_=idxf[:])
            # wrapped-16 index layout (see SWDGE): built with stream-shuffles;
            # the 32->128 replication runs on the Scalar engine
            wr = wr_pool.tile([128, byk * 8], I16, tag="wr")
            wr_v = wr[:].rearrange("p (yk q) -> p yk q", yk=byk, q=8)
            for qj in range(4):
                for qh in range(2):
                    mask = [16 * qh + (p % 16) for p in range(32)]
                    nc.vector.stream_shuffle(
                        out=wr_v[0:32, :, 2 * qj + qh],
                        in_=scr_i16[32 * qj:32 * (qj + 1), :], mask=mask)
            for qt in range(1, 4):
                nc.scalar.copy(
                    out=wr[32 * qt:32 * (qt + 1), :], in_=wr[0:32, :])
            wr_tiles[b] = wr

        # ---- stage B (per chunk): gather + weight + corner adds (DVE),
        #      block transposes + conv matmul (PE), store ----
        def emit_chunk(c):
            b, lc = c // cpb, c % cpb
            g = g_pool.tile([128, CHUNK * KK * EROW], BF16, tag="g")
            g_m = g[:].rearrange("p (m e) -> p m e", m=CHUNK * KK, e=EROW)
            nidx_g = 128 * cyk
            nc.gpsimd.dma_gather(
                out_ap=g_m, in_ap=table[:],
                idxs_ap=wr_tiles[b][:, lc * cyk * 8:(lc + 1) * cyk * 8],
                num_idxs=nidx_g, num_idxs_reg=nidx_g, elem_size=EROW,
                single_packet=False, queue_num=c % 4)
            # bilinear weight (corner-minor rows: c outer, f inner 4-contig);
            # the corner sum is absorbed by the f-replicated conv weights
            g_v = g[:].rearrange(
                "p (y k e f) -> p y k e f", y=CHUNK, k=KK, e=C, f=4)
            wf_v = wf_tiles[b][:].rearrange(
                "p (y k f) -> p y k f", y=brow, k=KK, f=4)
            wfb = wf_v[:, lc * CHUNK:(lc + 1) * CHUNK, :, None, :].to_broadcast(
                (128, CHUNK, KK, C, 4))
            nc.vector.tensor_tensor(out=g_v, in0=g_v, in1=wfb, op=OP.mult)

            # PE transpose of the weighted (c-half, f) 128-blocks straight
            # out of the gather tile; tile t=2k+h, bias rides tile 18
            st = st_pool.tile([128, CHUNK * KTILES * 128], BF16, tag="st")
            st_y = st[:].rearrange(
                "p (y j e) -> p y j e", y=CHUNK, j=KTILES, e=128)
            nc.scalar.memzero(st_y[:, :, KTILES - 1, :])
            for y in range(CHUNK):
                for grp in range(4):
                    pst = pst_pool.tile([128, 5 * 128], BF16, tag="pst")
                    nblk = 5 if grp < 3 else 3
                    for j5 in range(nblk):
                        j = grp * 5 + j5
                        k, hh = j // 2, j % 2
                        nc.tensor.transpose(
                            out=pst[:, j5 * 128:(j5 + 1) * 128],
                            in_=g_v[:, y, k, 32 * hh:32 * hh + 32, :].rearrange(
                                "p e f -> p (e f)"),
                            identity=idt[:])
                    if grp == 3:
                        nc.tensor.transpose(
                            out=pst[0:1, 3 * 128:4 * 128],
                            in_=onesc[:, 0:1], identity=idt[:])
                    dst = st_y[:, y, grp * 5:grp * 5 + nblk, :].rearrange(
                        "p j e -> p (j e)")
                    nc.scalar.copy(out=dst, in_=pst[:, 0:nblk * 128])
                    if grp == 3:
                        nc.scalar.copy(
                            out=st_y[0:1, y, KTILES - 1, :],
                            in_=pst[0:1, 3 * 128:4 * 128])

            ps2 = psum2_pool.tile([O, CHUNK * W], F32, tag="ps2")
            for j in range(KTILES):
                rhs = st_y[:, :, j, :]
                nc.tensor.matmul(
                    out=ps2[:], lhsT=w2t_v[:, j, :], rhs=rhs,
                    start=(j == 0), stop=(j == KTILES - 1),
                )
            ob = o_pool.tile([O, CHUNK * W], F32, tag="ob")
            nc.scalar.copy(out=ob[:], in_=ps2[:])
            nc.sync.dma_start(
                out=out[:, c * CHUNK * W:(c + 1) * CHUNK * W], in_=ob[:])

        for b in range(nbatch):
            emit_stage_a(b)
        for c in range(nchunks):
            emit_chunk(c)
        ctx.close()
    nc.compile()
    return nc


# ---------------- host side ----------------

def host_prepare(x, off_w, off_b, weight, bias, nrows=ROWS):
    """Build per-core input maps."""
    B = x.shape[0]
    x = np.asarray(x, np.float32)
    # padded image for offset conv, bf16, [B, C, H+2, W+2]
    xpad = np.pad(x, ((0, 0), (0, 0), (1, 1), (1, 1))).astype(ml_dtypes.bfloat16)
    # patch table per image: padded-by-2 channels-last
    xp2 = np.pad(x, ((0, 0), (0, 0), (PADT, PADT + 1), (PADT, PADT + 1)))
    # [B, PW+1, PW+1, C] channels-last
    xcl = xp2.transpose(0, 2, 3, 1)
    # table[q=(yp*PW+xp)] corner-minor: per channel the 4 corners
    # [x(y,x,c), x(y,x+1,c), x(y+1,x,c), x(y+1,x+1,c)]
    tables = []
    for b in range(B):
        t = np.empty((PW, PW, C, 4), np.float32)
        t[:, :, :, 0] = xcl[b, :PW, :PW]
        t[:, :, :, 1] = xcl[b, :PW, 1:PW + 1]
        t[:, :, :, 2] = xcl[b, 1:PW + 1, :PW]
        t[:, :, :, 3] = xcl[b, 1:PW + 1, 1:PW + 1]
        tables.append(t.reshape(NTAB, EROW).astype(ml_dtypes.bfloat16))

    # offset conv weights: channel perm [dy taps 0..8, dx taps 0..8]
    perm = [2 * k for k in range(KK)] + [2 * k + 1 for k in range(KK)]
    w_off_p = np.asarray(off_w, np.float32)[perm]          # [18, C, 3, 3]
    # lhsT per tap: tap k = dy*3+dx -> [C, 18]
    w_off_t = np.empty((KK, C, 18), np.float32)
    for k in range(KK):
        dy, dx = k // 3, k % 3
        w_off_t[k] = w_off_p[:, :, dy, dx].T               # [C, 18]
    w_off_t = w_off_t.astype(ml_dtypes.bfloat16)
    b_off_p = np.asarray(off_b, np.float32)[perm].reshape(18, 1)

    # main weights: tile t=2k+h row r -> weight[o, 32h + r//4, k]
    # (f-replicated, absorbing the corner sum); tile 18 row 0 = bias
    wgt = np.asarray(weight, np.float32).reshape(O, C, KK)
    kco = wgt.transpose(2, 1, 0)                            # [k, c, O]
    w2f = np.zeros((KTILES, 128, O), np.float32)
    for k in range(KK):
        for hh in range(2):
            w2f[2 * k + hh] = np.repeat(kco[k, 32 * hh:32 * hh + 32], 4, axis=0)
    w2f[18, 0] = np.asarray(bias, np.float32)
    w2f = w2f.astype(ml_dtypes.bfloat16)

    ky, kx = np.meshgrid(np.arange(3), np.arange(3), indexing="ij")
    ky = ky.reshape(KK).astype(np.float32)
    kx = kx.reshape(KK).astype(np.float32)
    iotax = np.arange(128, dtype=np.float32).reshape(128, 1)
    identity = np.eye(128, dtype=ml_dtypes.bfloat16)

    in_maps = []
    for core in range(8):
        b, hh = core // 2, core % 2
        y0 = hh * 64
        rows = np.arange(y0, y0 + nrows, dtype=np.float32)
        cgy = (rows[:, None] + ky[None, :] - 1.0).reshape(1, nrows * KK)
        cgx = np.broadcast_to(kx[None, :] - 1.0, (nrows, KK)).reshape(1, nrows * KK)
        cgrid = np.concatenate([cgy, cgx], 0).astype(np.float32)
        in_maps.append({
            "xband": np.ascontiguousarray(xpad[b, :, y0:y0 + nrows + 2, :]),
            "table": tables[b],
            "w_off": w_off_t,
            "b_off": b_off_p,
            "w2": w2f,
            "cgrid": cgrid,
            "iotax": iotax,
            "ident": identity,
        })
    return in_maps


def assemble(outs, nrows=ROWS):
    """outs: list of 8 dicts with 'out' [O, nrows*W] -> [4, O, H, W]"""
    full = np.zeros((4, O, H, W), np.float32)
    for core, om in enumerate(outs):
        b, hh = core // 2, core % 2
        full[b, :, hh * 64:hh * 64 + nrows] = om["out"].reshape(O, nrows, W)
    return full


_CACHE = {}


def kernel(x, off_w, off_b, weight, bias):
    if "nc" not in _CACHE:
        _CACHE["nc"] = build_program()
    nc = _CACHE["nc"]
    in_maps = host_prepare(x, off_w, off_b, weight, bias)
    from concourse.bass_utils import run_bass_kernel_spmd
    res = run_bass_kernel_spmd(nc, in_maps, core_ids=list(range(8)))
    return assemble(res.results)


# revision 44
# speedup vs baseline: 1.3707x; 1.3707x over previous
"""Trainium2 Bass kernel for DeformableConvBlock (B=4, C=64, H=W=128, K=3).

Self-contained: builds an SPMD Bass/Tile program for 8 NeuronCores.
Core c handles image c//2, output-row half c%2 (data-parallel over
batch x row-halves).

v5: gather-paced pipeline with the DMA rings reserved for the SWDGE
patch gathers. DVE does the bilinear weight multiply (corner-minor
table rows [64 ch x 4 corners] - the fast broadcast pattern) plus two
corner adds; the PE transposes the reduced 576-row contraction blocks
(matmul is_transpose) with Scalar PSUM->SBUF copies, and the conv
matmul contracts 5 x 128 tiles. The conv bias rides a constant-1 pad
row of the contraction. Index-replication copies and the offset cast
run on the Scalar engine to keep the DVE light. Out-of-bounds corners
are handled by index-clamping into the table's zero-padded border
(exactly equivalent to the reference's validity masking), so the prep
needs no mask ops.

kernel(**inputs) takes the full unsharded numpy inputs and returns the
full [4, 64, 128, 128] float32 output.
"""
from contextlib import ExitStack

import numpy as np
import ml_dtypes

import concourse.bacc as bacc
import concourse.bass as bass
import concourse.mybir as mybir
from concourse.tile import TileContext
from concourse.vector_clock import ScopedClock, VectorClock

F32 = mybir.dt.float32
BF16 = mybir.dt.bfloat16
I32 = mybir.dt.int32
I16 = mybir.dt.int16
AF = mybir.ActivationFunctionType
OP = mybir.AluOpType

H = W = 128
C = 64
O = 64
KK = 9
ROWS = 64            # output rows per core
NPX = ROWS * W       # 8192
PADT = 2             # table padding on each side
PW = W + 2 * PADT    # 132
NTAB = PW * PW       # 17424 table rows
EROW = 4 * C         # 256 elems per table row
KTILES = 5           # 640 = 5*128 contraction rows (577 used incl bias)
CHUNK = 2            # rows per gather/matmul chunk (32 calls keep all 4
                     # SWDGE queues saturated: 194us vs 270us at 4 rows)
CCH = 4              # rows per offset-conv chunk (PSUM 512-col limit)
BATCH = 16           # rows per prep batch
BIG = 1024.0         # floor-trick offset


class TileContextSplitDrain(TileContext):
    """Stock epilogue emits one Drain with one wait per outstanding proc;
    this walrus rejects >1 sync wait per instruction, so emit one Drain
    per proc instead."""

    def _drain_and_barrier(self, tick_clock, wait_clock):
        gc = tick_clock.global_clock
        nprocs = len(gc)
        emitted = False
        for p in range(nprocs):
            t = gc[p]
            if t <= 0:
                continue
            vec = [0] * nprocs
            vec[p] = t
            drain_inst = self.nc.sync.drain()
            wait_clock.add_sem_waits(
                drain_inst.ins, ScopedClock({None: VectorClock(vec)})
            )
            si = drain_inst.ins.sync_info
            assert si is None or len(si.on_wait) <= 1
            emitted = True
        if not emitted:
            self.nc.sync.drain()
        self.nc.all_engine_barrier()
        assert self.sems is not None
        popped = self.nc._tile_sem_poison_stack.pop()
        assert popped is self._sem_poison
        self.nc.clear_and_free_semaphores(list(self.sems.allocated().values()))
        self.nc.all_engine_barrier()


def build_program(nrows=ROWS, g_bufs=8):
    """Build the SPMD Bass program. nrows<=64 shrinks work for sim tests."""
    npx = nrows * W
    nchunks = nrows // CHUNK
    nbatch = max(1, nrows // BATCH)
    brow = min(BATCH, nrows)              # rows per prep batch
    byk = brow * KK                       # idx cols per batch
    cyk = CHUNK * KK                      # idx cols per chunk
    cpb = brow // CHUNK                   # gather chunks per batch
    ccb = brow // CCH                     # conv chunks per batch
    band = brow + 2                       # xband rows per batch

    nc = bacc.Bacc(num_swdge_queues=4)
    xband = nc.dram_tensor("xband", [C, nrows + 2, W + 2], BF16, kind="ExternalInput")
    table = nc.dram_tensor("table", [NTAB, EROW], BF16, kind="ExternalInput")
    w_off = nc.dram_tensor("w_off", [KK, C, 18], BF16, kind="ExternalInput")
    b_off = nc.dram_tensor("b_off", [18, 1], F32, kind="ExternalInput")
    w2 = nc.dram_tensor("w2", [KTILES, 128, O], BF16, kind="ExternalInput")
    cgrid = nc.dram_tensor("cgrid", [2, nrows * KK], F32, kind="ExternalInput")
    iotax = nc.dram_tensor("iotax", [128, 1], F32, kind="ExternalInput")
    ident = nc.dram_tensor("ident", [128, 128], BF16, kind="ExternalInput")
    out = nc.dram_tensor("out", [O, npx], F32, kind="ExternalOutput")

    ctx = ExitStack()
    with TileContextSplitDrain(nc) as tc:
        const_pool = ctx.enter_context(tc.tile_pool(name="const", bufs=1))
        scr_pool = ctx.enter_context(tc.tile_pool(name="scr", bufs=1))
        xbb_pool = ctx.enter_context(tc.tile_pool(name="xbb", bufs=2))
        grid_pool = ctx.enter_context(tc.tile_pool(name="grid", bufs=2))
        oc_pool = ctx.enter_context(tc.tile_pool(name="oc", bufs=1))
        op_pool = ctx.enter_context(tc.tile_pool(name="op", bufs=2))
        opf_pool = ctx.enter_context(tc.tile_pool(name="opf", bufs=1))
        wf_pool = ctx.enter_context(tc.tile_pool(name="wf", bufs=4))
        wr_pool = ctx.enter_context(tc.tile_pool(name="wr", bufs=4))
        g_pool = ctx.enter_context(tc.tile_pool(name="g", bufs=g_bufs))
        s_pool = ctx.enter_context(tc.tile_pool(name="s", bufs=2))
        st_pool = ctx.enter_context(tc.tile_pool(name="st", bufs=2))
        o_pool = ctx.enter_context(tc.tile_pool(name="o", bufs=2))
        psum_pool = ctx.enter_context(tc.tile_pool(name="ps", bufs=2, space="PSUM"))
        pot_pool = ctx.enter_context(tc.tile_pool(name="pot", bufs=1, space="PSUM"))
        pst_pool = ctx.enter_context(tc.tile_pool(name="pst", bufs=2, space="PSUM"))
        psum2_pool = ctx.enter_context(tc.tile_pool(name="ps2", bufs=2, space="PSUM"))

        # ---- loads ----
        wof = const_pool.tile([C, KK * 18], BF16)
        wof_v = wof[:].rearrange("c (k e) -> c k e", k=KK, e=18)
        nc.sync.dma_start(out=wof_v, in_=w_off[:].rearrange("k c e -> c k e"))

        bof = const_pool.tile([18, 1], F32)
        nc.sync.dma_start(out=bof[:], in_=b_off[:])

        w2t = const_pool.tile([128, KTILES * O], BF16)
        w2t_v = w2t[:].rearrange("p (j e) -> p j e", j=KTILES, e=O)
        nc.sync.dma_start(out=w2t_v, in_=w2[:].rearrange("j p e -> p j e"))

        idt = const_pool.tile([128, 128], BF16)
        nc.sync.dma_start(out=idt[:], in_=ident[:])

        iox = const_pool.tile([128, 1], F32)
        nc.sync.dma_start(out=iox[:], in_=iotax[:])

        onesc = const_pool.tile([128, CHUNK], BF16)
        nc.vector.memset(onesc[:], 1.0)

        # prep scratch (live range = within one batch; DVE runs batches
        # sequentially so one buffer per name suffices)
        scr = {}
        for nm in ("ya", "yb", "yd", "ye", "xa", "xc", "xd", "xe"):
            scr[nm] = scr_pool.tile([128, byk], F32, name=f"scr_{nm}")
        scr_i32 = scr_pool.tile([128, byk], I32, name="scr_i32")
        scr_i16 = scr_pool.tile([128, byk], I16, name="scr_i16")

        wf_tiles = [None] * nbatch
        wr_tiles = [None] * nbatch

        # ---- stage A (per batch): offset conv + transpose + prep ----
        def emit_stage_a(b):
            xbb = xbb_pool.tile([C, band * (W + 2)], BF16, tag="xbb")
            nc.sync.dma_start(
                out=xbb[:],
                in_=xband[:, b * brow:b * brow + band, :].rearrange(
                    "c h w -> c (h w)"))
            xbb_v = xbb[:].rearrange("c (h w) -> c h w", h=band, w=W + 2)

            cyb = grid_pool.tile([128, byk], F32, tag="cy")
            nc.sync.dma_start(
                out=cyb[:],
                in_=cgrid[0:1, b * byk:(b + 1) * byk].to_broadcast((128, byk)))
            cxb = grid_pool.tile([128, byk], F32, tag="cx")
            nc.sync.dma_start(
                out=cxb[:],
                in_=cgrid[1:2, b * byk:(b + 1) * byk].to_broadcast((128, byk)))

            # offset conv, CCH rows per psum tile
            ocb = oc_pool.tile([32, brow * W], BF16, tag="ocb")
            if b == 0:
                nc.vector.memset(ocb[:], 0.0)
            for cc in range(ccb):
                r0 = cc * CCH
                ps = psum_pool.tile([18, CCH * W], F32, tag="ps1")
                for k in range(KK):
                    dy, dx = k // 3, k % 3
                    rhs = xbb_v[:, r0 + dy:r0 + dy + CCH, dx:dx + W]
                    nc.tensor.matmul(
                        out=ps[:], lhsT=wof_v[:, k, :], rhs=rhs,
                        start=(k == 0), stop=(k == KK - 1),
                    )
                nc.scalar.activation(
                    out=ocb[:18, cc * CCH * W:(cc + 1) * CCH * W], in_=ps[:],
                    func=AF.Identity, bias=bof[:],
                )
            # offsets to pixel-major via PE transpose:
            # [32, brow*W] -> [128, brow, 32]
            pot = pot_pool.tile([128, brow * 32], BF16, tag="pot")
            for bi in range(brow):
                nc.tensor.transpose(
                    out=pot[:, bi * 32:(bi + 1) * 32],
                    in_=ocb[:, bi * W:(bi + 1) * W],
                    identity=idt[0:32, 0:32])
            opb = op_pool.tile([128, brow * 32], BF16, tag="opb")
            nc.scalar.copy(out=opb[:], in_=pot[:])
            opb3 = opb[:].rearrange("p (y e) -> p y e", y=brow, e=32)

            opf = opf_pool.tile([128, brow * 18], F32, tag="opf")
            opf_v = opf[:].rearrange("p (y e) -> p y e", y=brow, e=18)
            nc.scalar.copy(out=opf_v, in_=opb3[:, :, 0:18])

            for axis in (0, 1):  # 0: y, 1: x
                a, bb, d, e = (
                    (scr["ya"], scr["yb"], scr["yd"], scr["ye"]) if axis == 0
                    else (scr["xa"], scr["xc"], scr["xd"], scr["xe"]))
                i0 = scr_i32
                off_src = opf_v[:, :, axis * 9:axis * 9 + 9]
                grid = cyb if axis == 0 else cxb
                s3 = a[:].rearrange("p (y k) -> p y k", y=brow, k=KK)
                grid3 = grid[:].rearrange("p (y k) -> p y k", y=brow, k=KK)
                nc.vector.tensor_tensor(out=s3, in0=off_src, in1=grid3, op=OP.add)
                if axis == 1:
                    nc.vector.tensor_tensor(
                        out=a[:], in0=a[:],
                        in1=iox[:].to_broadcast((128, byk)), op=OP.add)
                # floor via +BIG truncation: d = trunc(a+BIG)-BIG, then
                # correct for convert rounding mode (floor = t - (a < t))
                nc.vector.tensor_scalar_add(out=bb[:], in0=a[:], scalar1=BIG)
                nc.vector.tensor_copy(out=i0[:], in_=bb[:])
                nc.vector.tensor_copy(out=d[:], in_=i0[:])
                nc.vector.tensor_scalar_add(out=d[:], in0=d[:], scalar1=-BIG)
                nc.vector.tensor_tensor(out=e[:], in0=a[:], in1=d[:], op=OP.is_lt)
                nc.vector.tensor_tensor(out=d[:], in0=d[:], in1=e[:], op=OP.subtract)
                # fractions: b = frac, a = 1-frac; e = clamped floor for idx
                nc.vector.tensor_tensor(out=bb[:], in0=a[:], in1=d[:], op=OP.subtract)
                nc.vector.tensor_scalar(
                    out=a[:], in0=bb[:], scalar1=-1.0, scalar2=1.0,
                    op0=OP.mult, op1=OP.add)
                nc.vector.tensor_scalar(
                    out=e[:], in0=d[:], scalar1=-2.0, scalar2=128.0,
                    op0=OP.max, op1=OP.min)

            # wf[...,f=2i+j] = wy_i * wx_j   (y: a=w0 b=w1 / x: a=w0 c=w1)
            wf = wf_pool.tile([128, brow * 36], BF16, tag="wf")
            wf_v = wf[:].rearrange("p (y k f) -> p y k f", y=brow, k=KK, f=4)
            for i, wy in enumerate((scr["ya"], scr["yb"])):
                for j, wx in enumerate((scr["xa"], scr["xc"])):
                    dst = wf_v[:, :, :, 2 * i + j].rearrange("p y k -> p (y k)")
                    nc.vector.tensor_tensor(out=dst, in0=wy[:], in1=wx[:], op=OP.mult)
            wf_tiles[b] = wf

            # idx = (yc+2)*132 + (xc+2); yc = scr.ye, xc = scr.xe
            idxf = scr["yd"]
            nc.vector.tensor_scalar(
                out=idxf[:], in0=scr["ye"][:], scalar1=float(PW),
                scalar2=float(PADT * PW + PADT), op0=OP.mult, op1=OP.add)
            nc.vector.tensor_tensor(out=idxf[:], in0=idxf[:], in1=scr["xe"][:], op=OP.add)
            nc.vector.tensor_copy(out=scr_i16[:], in_=idxf[:])
            # wrapped-16 index layout (see SWDGE): built with stream-shuffles;
            # the 32->128 replication runs on the Scalar engine
            wr = wr_pool.tile([128, byk * 8], I16, tag="wr")
            wr_v = wr[:].rearrange("p (yk q) -> p yk q", yk=byk, q=8)
            for qj in range(4):
                for qh in range(2):
                    mask = [16 * qh + (p % 16) for p in range(32)]
                    nc.vector.stream_shuffle(
                        out=wr_v[0:32, :, 2 * qj + qh],
                        in_=scr_i16[32 * qj:32 * (qj + 1), :], mask=mask)
            for qt in range(1, 4):
                nc.scalar.copy(
                    out=wr[32 * qt:32 * (qt + 1), :], in_=wr[0:32, :])
            wr_tiles[b] = wr

        # ---- stage B (per chunk): gather + weight + corner adds (DVE),
        #      block transposes + conv matmul (PE), store ----
        def emit_chunk(c):
            b, lc = c // cpb, c % cpb
            g = g_pool.tile([128, CHUNK * KK * EROW], BF16, tag="g")
            g_m = g[:].rearrange("p (m e) -> p m e", m=CHUNK * KK, e=EROW)
            nidx_g = 128 * cyk
            nc.gpsimd.dma_gather(
                out_ap=g_m, in_ap=table[:],
                idxs_ap=wr_tiles[b][:, lc * cyk * 8:(lc + 1) * cyk * 8],
                num_idxs=nidx_g, num_idxs_reg=nidx_g, elem_size=EROW,
                single_packet=False, queue_num=c % 4)
            # bilinear weight (corner-minor rows: c outer, f inner 4-contig)
            g_v = g[:].rearrange(
                "p (y k e f) -> p y k e f", y=CHUNK, k=KK, e=C, f=4)
            wf_v = wf_tiles[b][:].rearrange(
                "p (y k f) -> p y k f", y=brow, k=KK, f=4)
            wfb = wf_v[:, lc * CHUNK:(lc + 1) * CHUNK, :, None, :].to_broadcast(
                (128, CHUNK, KK, C, 4))
            nc.vector.tensor_tensor(out=g_v, in0=g_v, in1=wfb, op=OP.mult)
            nc.vector.tensor_tensor(
                out=g_v[:, :, :, :, 0:2], in0=g_v[:, :, :, :, 0:2],
                in1=g_v[:, :, :, :, 2:4], op=OP.add)
            s8 = s_pool.tile([128, CHUNK * KTILES * 128], BF16, tag="s8")
            s8_v = s8[:].rearrange("p (y e) -> p y e", y=CHUNK, e=KTILES * 128)
            # pad of the contraction: zeros then bias-ones col (Scalar engine)
            nc.scalar.memzero(s8_v[:, :, KK * C:])
            nc.scalar.copy(out=s8_v[:, :, KK * C], in_=onesc[:])
            sdst = s8_v[:, :, 0:KK * C].rearrange(
                "p y (k e) -> p y k e", k=KK, e=C)
            nc.vector.tensor_tensor(
                out=sdst, in0=g_v[:, :, :, :, 0], in1=g_v[:, :, :, :, 1],
                op=OP.add)

            # PE transpose of s8 128-blocks, 5 per psum group:
            # st[pe, y, j, x] = s8[x, y, j*128+pe]
            st = st_pool.tile([128, CHUNK * KTILES * 128], BF16, tag="st")
            for grp in range(CHUNK * KTILES // 5):
                pst = pst_pool.tile([128, 5 * 128], BF16, tag="pst")
                for j5 in range(5):
                    blk = grp * 5 + j5
                    y, j = blk // KTILES, blk % KTILES
                    nc.tensor.transpose(
                        out=pst[:, j5 * 128:(j5 + 1) * 128],
                        in_=s8_v[:, y, j * 128:(j + 1) * 128],
                        identity=idt[:])
                nc.scalar.copy(
                    out=st[:, grp * 640:(grp + 1) * 640], in_=pst[:])
            st_y = st[:].rearrange(
                "p (y j e) -> p y j e", y=CHUNK, j=KTILES, e=128)

            ps2 = psum2_pool.tile([O, CHUNK * W], F32, tag="ps2")
            for j in range(KTILES):
                rhs = st_y[:, :, j, :]
                nc.tensor.matmul(
                    out=ps2[:], lhsT=w2t_v[:, j, :], rhs=rhs,
                    start=(j == 0), stop=(j == KTILES - 1),
                )
            ob = o_pool.tile([O, CHUNK * W], F32, tag="ob")
            nc.scalar.copy(out=ob[:], in_=ps2[:])
            nc.sync.dma_start(
                out=out[:, c * CHUNK * W:(c + 1) * CHUNK * W], in_=ob[:])

        for b in range(nbatch):
            emit_stage_a(b)
        for c in range(nchunks):
            emit_chunk(c)
        ctx.close()
    nc.compile()
    return nc


# ---------------- host side ----------------

def host_prepare(x, off_w, off_b, weight, bias, nrows=ROWS):
    """Build per-core input maps."""
    B = x.shape[0]
    x = np.asarray(x, np.float32)
    # padded image for offset conv, bf16, [B, C, H+2, W+2]
    xpad = np.pad(x, ((0, 0), (0, 0), (1, 1), (1, 1))).astype(ml_dtypes.bfloat16)
    # patch table per image: padded-by-2 channels-last
    xp2 = np.pad(x, ((0, 0), (0, 0), (PADT, PADT + 1), (PADT, PADT + 1)))
    # [B, PW+1, PW+1, C] channels-last
    xcl = xp2.transpose(0, 2, 3, 1)
    # table[q=(yp*PW+xp)] corner-minor: per channel the 4 corners
    # [x(y,x,c), x(y,x+1,c), x(y+1,x,c), x(y+1,x+1,c)]
    tables = []
    for b in range(B):
        t = np.empty((PW, PW, C, 4), np.float32)
        t[:, :, :, 0] = xcl[b, :PW, :PW]
        t[:, :, :, 1] = xcl[b, :PW, 1:PW + 1]
        t[:, :, :, 2] = xcl[b, 1:PW + 1, :PW]
        t[:, :, :, 3] = xcl[b, 1:PW + 1, 1:PW + 1]
        tables.append(t.reshape(NTAB, EROW).astype(ml_dtypes.bfloat16))

    # offset conv weights: channel perm [dy taps 0..8, dx taps 0..8]
    perm = [2 * k for k in range(KK)] + [2 * k + 1 for k in range(KK)]
    w_off_p = np.asarray(off_w, np.float32)[perm]          # [18, C, 3, 3]
    # lhsT per tap: tap k = dy*3+dx -> [C, 18]
    w_off_t = np.empty((KK, C, 18), np.float32)
    for k in range(KK):
        dy, dx = k // 3, k % 3
        w_off_t[k] = w_off_p[:, :, dy, dx].T               # [C, 18]
    w_off_t = w_off_t.astype(ml_dtypes.bfloat16)
    b_off_p = np.asarray(off_b, np.float32)[perm].reshape(18, 1)

    # main weights: W2[(k,c), o] = weight[o, c, k], padded to 640 rows;
    # row 576 carries the bias (the rhs pad column is memset to 1)
    wgt = np.asarray(weight, np.float32).reshape(O, C, KK)
    w2f = np.zeros((KTILES * 128, O), np.float32)
    kc = wgt.transpose(2, 1, 0).reshape(KK * C, O)          # [(k,c), O]
    w2f[:KK * C] = kc
    w2f[KK * C] = np.asarray(bias, np.float32)
    w2f = w2f.reshape(KTILES, 128, O).astype(ml_dtypes.bfloat16)

    ky, kx = np.meshgrid(np.arange(3), np.arange(3), indexing="ij")
    ky = ky.reshape(KK).astype(np.float32)
    kx = kx.reshape(KK).astype(np.float32)
    iotax = np.arange(128, dtype=np.float32).reshape(128, 1)
    identity = np.eye(128, dtype=ml_dtypes.bfloat16)

    in_maps = []
    for core in range(8):
        b, hh = core // 2, core % 2
        y0 = hh * 64
        rows = np.arange(y0, y0 + nrows, dtype=np.float32)
        cgy = (rows[:, None] + ky[None, :] - 1.0).reshape(1, nrows * KK)
        cgx = np.broadcast_to(kx[None, :] - 1.0, (nrows, KK)).reshape(1, nrows * KK)
        cgrid = np.concatenate([cgy, cgx], 0).astype(np.float32)
        in_maps.append({
            "xband": np.ascontiguousarray(xpad[b, :, y0:y0 + nrows + 2, :]),
            "table": tables[b],
            "w_off": w_off_t,
            "b_off": b_off_p,
            "w2": w2f,
            "cgrid": cgrid,
            "iotax": iotax,
            "ident": identity,
        })
    return in_maps


def assemble(outs, nrows=ROWS):
    """outs: list of 8 dicts with 'out' [O, nrows*W] -> [4, O, H, W]"""
    full = np.zeros((4, O, H, W), np.float32)
    for core, om in enumerate(outs):
        b, hh = core // 2, core % 2
        full[b, :, hh * 64:hh * 64 + nrows] = om["out"].reshape(O, nrows, W)
    return full


_CACHE = {}


def kernel(x, off_w, off_b, weight, bias):
    if "nc" not in _CACHE:
        _CACHE["nc"] = build_program()
    nc = _CACHE["nc"]
    in_maps = host_prepare(x, off_w, off_b, weight, bias)
    from concourse.bass_utils import run_bass_kernel_spmd
    res = run_bass_kernel_spmd(nc, in_maps, core_ids=list(range(8)))
    return assemble(res.results)
